# revision 1
# baseline (speedup 1.0000x reference)
"""Trainium2 Bass kernel for the DISL loss (nn_DISL_Loss).

Strategy (data-parallel over batch, 8 cores):
  Pass 1 (device): per-core contraction over its 2048 (b,t) rows:
    G_A = O_A^T V, G_F = O_F^T V  [512,1024] partials, column sums of
    squares of V/O_A/O_F (for sim normalization), and the triplet
    weighted row-sum T = W^T vaf_satt [3,1024] partials.
  Host: all-reduce partials, normalize -> sim, greedy unique assignment
    (tiny, sequential), build one-hot permutation matrices P_A, P_F.
  Pass 2 (device): per-core gathered-column dots via one-hot matmul:
    Ap = O_A @ P_A, Fp = O_F @ P_F; per-row n1=<V,Ap>, n2=<V,Fp>,
    n3=<Ap,Fp> and row sums of squares of V/O_A/O_F.
  Host: cos/CE/BCE/triplet final combine (small tensors only).

Key identity: ext is a permutation of 0..1023, so ||pA_row|| equals
||O_A_row|| and no gathered norms are needed.
"""

import numpy as np
import ml_dtypes

B, T, M, OM = 64, 256, 1024, 512
N_CORES = 8
SPC = B // N_CORES          # samples per core
RPC = SPC * T               # rows per core
P = 128

_prog_cache = {}


# ---------------------------------------------------------------- pass 1
def _build_pass1(rows, g_f32r=False):
    from concourse import bacc, mybir
    from concourse.tile import TileContext

    f32 = mybir.dt.float32
    gdt = mybir.dt.float32r if g_f32r else f32
    kt = rows // P
    ST = 4 if kt % 4 == 0 else (2 if kt % 2 == 0 else 1)
    NS = kt // ST
    nc = bacc.Bacc()
    v_d = nc.declare_dram_parameter("v", [rows, M], gdt, isOutput=False)
    oa_d = nc.declare_dram_parameter("oa", [rows, OM], gdt, isOutput=False)
    of_d = nc.declare_dram_parameter("of", [rows, OM], gdt, isOutput=False)
    vaf_d = nc.declare_dram_parameter("vaf", [rows, M], f32, isOutput=False)
    w_d = nc.declare_dram_parameter("w", [rows, 4], f32, isOutput=False)
    ga_d = nc.declare_dram_parameter("ga", [OM, M], f32, isOutput=True)
    gf_d = nc.declare_dram_parameter("gf", [OM, M], f32, isOutput=True)
    csq_d = nc.declare_dram_parameter("csq", [3, M], f32, isOutput=True)
    tm_d = nc.declare_dram_parameter("tm", [4, M], f32, isOutput=True)

    def sup(dram, s):  # super-tile view: ST row-tiles in one DMA
        return dram[s * ST * P:(s + 1) * ST * P, :].rearrange(
            "(t p) m -> p t m", p=P)

    with TileContext(nc) as tc:
        with (
            tc.tile_pool(name="vres", bufs=NS) as vpool,
            tc.tile_pool(name="stream", bufs=2) as spool,
            tc.tile_pool(name="sq", bufs=2) as qpool,
            tc.tile_pool(name="acc", bufs=1) as apool,
            tc.tile_pool(name="cb", bufs=3) as cbpool,
            tc.tile_pool(name="const", bufs=1) as cpool,
        ):
            ones = cpool.tile([P, 1], f32, tag="ones")
            nc.vector.memset(ones[:], 1.0)
            accv = apool.tile([P, ST, M], f32, tag="accv")
            acca = apool.tile([P, ST, OM], f32, tag="acca")
            accf = apool.tile([P, ST, OM], f32, tag="accf")

            vtiles = []
            for s in range(NS):
                vt = vpool.tile([P, ST, M], gdt, tag="v", name=f"vres{s}")
                eng = nc.sync if s % 2 == 0 else nc.scalar
                eng.dma_start(vt[:], sup(v_d, s))
                vtiles.append(vt)

            # ---- phase A: G_A accumulation + squares of v, oa ----
            with tc.tile_pool(name="psA", bufs=8, space="PSUM") as ppa:
                gps = [ppa.tile([P, 512], f32, tag="g", name=f"gpsA{i}")
                       for i in range(8)]
                for s in range(NS):
                    oat = spool.tile([P, ST, OM], gdt, tag="oaf")
                    eng = nc.sync if s % 2 == 0 else nc.scalar
                    eng.dma_start(oat[:], sup(oa_d, s))
                    for j in range(ST):
                        for mc in range(4):
                            for nh in range(2):
                                nc.tensor.matmul(
                                    gps[mc * 2 + nh][:],
                                    lhsT=oat[:, j, mc * P:(mc + 1) * P],
                                    rhs=vtiles[s][:, j, nh * 512:(nh + 1) * 512],
                                    start=(s == 0 and j == 0),
                                    stop=(s == NS - 1 and j == ST - 1),
                                )
                    for j in range(ST):
                        if s == 0:
                            nc.vector.tensor_mul(accv[:, j, :],
                                                 vtiles[s][:, j, :].bitcast(f32),
                                                 vtiles[s][:, j, :].bitcast(f32))
                            nc.vector.tensor_mul(acca[:, j, :], oat[:, j, :].bitcast(f32),
                                                 oat[:, j, :].bitcast(f32))
                        else:
                            sqv = qpool.tile([P, M], f32, tag="sqv",
                                             name=f"sqv{s}_{j}")
                            nc.vector.tensor_mul(sqv[:], vtiles[s][:, j, :].bitcast(f32),
                                                 vtiles[s][:, j, :].bitcast(f32))
                            nc.vector.tensor_add(accv[:, j, :], accv[:, j, :],
                                                 sqv[:])
                            sqa = qpool.tile([P, OM], f32, tag="sqa",
                                             name=f"sqa{s}_{j}")
                            nc.vector.tensor_mul(sqa[:], oat[:, j, :].bitcast(f32),
                                                 oat[:, j, :].bitcast(f32))
                            nc.vector.tensor_add(acca[:, j, :], acca[:, j, :],
                                                 sqa[:])
                for mc in range(4):
                    cb = cbpool.tile([P, M], f32, tag="cb", name=f"cba{mc}")
                    nc.scalar.copy(cb[:, 0:512], gps[mc * 2][:])
                    nc.scalar.copy(cb[:, 512:M], gps[mc * 2 + 1][:])
                    eng = nc.sync if mc % 2 == 0 else nc.scalar
                    eng.dma_start(ga_d[mc * P:(mc + 1) * P, :], cb[:])

            # ---- phase B: G_F accumulation + squares of of ----
            with tc.tile_pool(name="psB", bufs=8, space="PSUM") as ppb:
                gps = [ppb.tile([P, 512], f32, tag="g", name=f"gpsB{i}")
                       for i in range(8)]
                for s in range(NS):
                    oft = spool.tile([P, ST, OM], gdt, tag="oaf", name=f"ofst{s}")
                    eng = nc.sync if s % 2 == 0 else nc.scalar
                    eng.dma_start(oft[:], sup(of_d, s))
                    for j in range(ST):
                        for mc in range(4):
                            for nh in range(2):
                                nc.tensor.matmul(
                                    gps[mc * 2 + nh][:],
                                    lhsT=oft[:, j, mc * P:(mc + 1) * P],
                                    rhs=vtiles[s][:, j, nh * 512:(nh + 1) * 512],
                                    start=(s == 0 and j == 0),
                                    stop=(s == NS - 1 and j == ST - 1),
                                )
                    for j in range(ST):
                        if s == 0:
                            nc.vector.tensor_mul(accf[:, j, :], oft[:, j, :].bitcast(f32),
                                                 oft[:, j, :].bitcast(f32))
                        else:
                            sqf = qpool.tile([P, OM], f32, tag="sqa",
                                             name=f"sqf{s}_{j}")
                            nc.vector.tensor_mul(sqf[:], oft[:, j, :].bitcast(f32),
                                                 oft[:, j, :].bitcast(f32))
                            nc.vector.tensor_add(accf[:, j, :], accf[:, j, :],
                                                 sqf[:])
                for mc in range(4):
                    cb = cbpool.tile([P, M], f32, tag="cb", name=f"cbf{mc}")
                    nc.scalar.copy(cb[:, 0:512], gps[mc * 2][:])
                    nc.scalar.copy(cb[:, 512:M], gps[mc * 2 + 1][:])
                    eng = nc.sync if mc % 2 == 0 else nc.scalar
                    eng.dma_start(gf_d[mc * P:(mc + 1) * P, :], cb[:])

            # ---- phase C: triplet weighted row-sum + csq reductions ----
            with tc.tile_pool(name="psC", bufs=1, space="PSUM") as ppc:
                wt_all = cpool.tile([P, kt, 4], f32, tag="w_all")
                nc.gpsimd.dma_start(
                    wt_all[:], w_d.rearrange("(k p) c -> p k c", p=P))
                tmps = ppc.tile([4, M], f32, tag="tm")
                for s in range(NS):
                    vft = spool.tile([P, ST, M], f32, tag="vaf")
                    eng = nc.sync if s % 2 == 0 else nc.scalar
                    eng.dma_start(vft[:], sup(vaf_d, s))
                    for j in range(ST):
                        for nh in range(2):
                            nc.tensor.matmul(
                                tmps[:, nh * 512:(nh + 1) * 512],
                                lhsT=wt_all[:, s * ST + j, :],
                                rhs=vft[:, j, nh * 512:(nh + 1) * 512],
                                start=(s == 0 and j == 0),
                                stop=(s == NS - 1 and j == ST - 1),
                            )
                cb = cbpool.tile([4, M], f32, tag="cbt")
                nc.scalar.copy(cb[:], tmps[:])
                nc.sync.dma_start(tm_d[:, :], cb[:])

                csqps = ppc.tile([1, M], f32, tag="csqv")
                for nh in range(2):
                    for t in range(ST):
                        nc.tensor.matmul(
                            csqps[:, nh * 512:(nh + 1) * 512],
                            lhsT=ones[:],
                            rhs=accv[:, t, nh * 512:(nh + 1) * 512],
                            start=(t == 0), stop=(t == ST - 1),
                        )
                cbv = cbpool.tile([1, M], f32, tag="cbv")
                nc.scalar.copy(cbv[:], csqps[:])
                nc.sync.dma_start(csq_d[0:1, :], cbv[:])

                csqps2 = ppc.tile([1, M], f32, tag="csqa")
                for t in range(ST):
                    nc.tensor.matmul(csqps2[:, 0:512], lhsT=ones[:],
                                     rhs=acca[:, t, :],
                                     start=(t == 0), stop=(t == ST - 1))
                    nc.tensor.matmul(csqps2[:, 512:M], lhsT=ones[:],
                                     rhs=accf[:, t, :],
                                     start=(t == 0), stop=(t == ST - 1))
                cba = cbpool.tile([1, M], f32, tag="cbv")
                nc.scalar.copy(cba[:], csqps2[:])
                nc.scalar.dma_start(csq_d[1:2, :], cba[:])
    nc.finalize()
    return nc


# ---------------------------------------------------------------- pass 2
def _build_pass2(rows):
    from concourse import bacc, mybir
    from concourse.tile import TileContext
    from concourse.masks import make_identity

    f32 = mybir.dt.float32
    bf16 = mybir.dt.bfloat16
    kt = rows // P
    ST = 4 if kt % 4 == 0 else (2 if kt % 2 == 0 else 1)
    NS = kt // ST
    nc = bacc.Bacc()
    v_d = nc.declare_dram_parameter("v", [rows, M], f32, isOutput=False)
    oa_d = nc.declare_dram_parameter("oa", [rows, OM], f32, isOutput=False)
    of_d = nc.declare_dram_parameter("of", [rows, OM], f32, isOutput=False)
    pa_d = nc.declare_dram_parameter("pa", [OM, M], bf16, isOutput=False)
    pf_d = nc.declare_dram_parameter("pf", [OM, M], bf16, isOutput=False)
    st_d = nc.declare_dram_parameter("st", [rows, 8], f32, isOutput=True)

    def sup(dram, s):
        return dram[s * ST * P:(s + 1) * ST * P, :].rearrange(
            "(t p) m -> p t m", p=P)

    with TileContext(nc) as tc:
        with (
            tc.tile_pool(name="const", bufs=1) as cpool,
            tc.tile_pool(name="ores", bufs=2 * NS) as opool,
            tc.tile_pool(name="obts", bufs=2 * kt) as tpool,
            tc.tile_pool(name="stream", bufs=2) as spool,
            tc.tile_pool(name="gsbp", bufs=4) as gpool,
            tc.tile_pool(name="scr", bufs=2) as qpool,
        ):
            X = mybir.AxisListType.X
            ident = cpool.tile([P, P], f32, tag="ident")
            make_identity(nc, ident[:])
            pa_sb = cpool.tile([P, 4, M], bf16, tag="pa")
            pf_sb = cpool.tile([P, 4, M], bf16, tag="pf")
            for i in range(4):
                eng = nc.sync if i % 2 == 0 else nc.scalar
                eng.dma_start(pa_sb[:, i, :], pa_d[i * P:(i + 1) * P, :])
                eng.dma_start(pf_sb[:, i, :], pf_d[i * P:(i + 1) * P, :])
            stat_all = cpool.tile([P, kt, 8], f32, tag="stat_all")
            nc.vector.memset(stat_all[:], 0.0)

            oa_res, of_res, obT = [], [], {}
            # ---- phase T: load O tensors, transpose all row-tiles ----
            with tc.tile_pool(name="ptr", bufs=6, space="PSUM") as ptr:
                for s in range(NS):
                    oat = opool.tile([P, ST, OM], f32, tag="o", name=f"oar{s}")
                    nc.sync.dma_start(oat[:], sup(oa_d, s))
                    oa_res.append(oat)
                    oft = opool.tile([P, ST, OM], f32, tag="o", name=f"ofr{s}")
                    nc.scalar.dma_start(oft[:], sup(of_d, s))
                    of_res.append(oft)
                    for j in range(ST):
                        k = s * ST + j
                        for name, ot in (("a", oat), ("f", oft)):
                            trp = ptr.tile([P, OM], f32, tag="tr",
                                           name=f"tr{k}{name}")
                            for i in range(4):
                                nc.tensor.transpose(
                                    trp[:, i * P:(i + 1) * P],
                                    ot[:, j, i * P:(i + 1) * P], ident[:])
                            ob = tpool.tile([P, OM], bf16, tag="obT",
                                            name=f"obT{k}{name}")
                            nc.vector.tensor_copy(ob[:], trp[:])
                            obT[(k, name)] = ob

            # ---- phase G: gather matmuls + per-row reductions ----
            with tc.tile_pool(name="pg", bufs=4, space="PSUM") as pgat:
                for s in range(NS):
                    vt = spool.tile([P, ST, M], f32, tag="v")
                    eng = nc.sync if s % 2 == 0 else nc.scalar
                    eng.dma_start(vt[:], sup(v_d, s))
                    for j in range(ST):
                        k = s * ST + j
                        gps = {}
                        for name, p_sb in (("a", pa_sb), ("f", pf_sb)):
                            ob = obT[(k, name)]
                            gp = pgat.tile([P, M], f32, tag="g",
                                           name=f"gp{k}{name}")
                            for i in range(4):
                                for nh in range(2):
                                    nc.tensor.matmul(
                                        gp[:, nh * 512:(nh + 1) * 512],
                                        lhsT=ob[:, i * P:(i + 1) * P],
                                        rhs=p_sb[:, i, nh * 512:(nh + 1) * 512],
                                        start=(i == 0),
                                        stop=(i == 3),
                                    )
                            gps[name] = gp
                        # stage only the A-gather in SBUF (DVE reads at most
                        # one PSUM operand per op)
                        apg = gpool.tile([P, M], f32, tag="gsb",
                                         name=f"gsb{k}")
                        nc.vector.tensor_copy(apg[:], gps["a"][:])
                        fpg = gps["f"]
                        vtj = vt[:, j, :]
                        # batched products -> two multi-slot reductions
                        prodA = qpool.tile([P, 3, M], f32, tag="prodA",
                                           name=f"prodA{k}")
                        nc.vector.tensor_mul(prodA[:, 0, :], vtj, apg[:])
                        nc.vector.tensor_mul(prodA[:, 1, :], vtj, fpg[:])
                        nc.vector.tensor_mul(prodA[:, 2, :], apg[:], fpg[:])
                        prodB = qpool.tile([P, 4, 512], f32, tag="prodB",
                                           name=f"prodB{k}")
                        nc.vector.tensor_mul(
                            prodB[:, 0:2, :].rearrange("p a b -> p (a b)"),
                            vtj, vtj)
                        nc.vector.tensor_mul(prodB[:, 2, :],
                                             oa_res[s][:, j, :],
                                             oa_res[s][:, j, :])
                        nc.vector.tensor_mul(prodB[:, 3, :],
                                             of_res[s][:, j, :],
                                             of_res[s][:, j, :])
                        st = stat_all[:, k, :]
                        nc.vector.reduce_sum(st[:, 0:3], prodA[:], axis=X)
                        nc.vector.reduce_sum(st[:, 3:7], prodB[:], axis=X)

            nc.sync.dma_start(
                st_d.rearrange("(k p) c -> p k c", p=P), stat_all[:])
    nc.finalize()
    return nc


# ---------------------------------------------------------------- host math
def _greedy_ext(sim):
    om, m = sim.shape
    used = np.zeros(m, dtype=bool)
    I = np.empty(om, dtype=np.int32)
    for r in range(om):
        row = np.where(used, -np.inf, sim[r])
        c = int(np.argmax(row))
        I[r] = c
        used[c] = True
    ext = np.empty(m, dtype=np.int32)
    ext[:om] = I
    ext[om:] = np.nonzero(~used)[0]
    return ext


def _triplet_weights(label, seq_len, vaf_avf):
    f32 = np.float32
    y = np.asarray(label).astype(np.int64)
    n_idx = np.nonzero(y == 0)[0]
    a_idx = np.nonzero(y == 1)[0]
    W = np.zeros((B, T, 4), f32)
    ar = np.arange(T)
    Nn, Na = len(n_idx), len(a_idx)
    if Nn and Na:
        for b in n_idx:
            L = int(seq_len[b])
            W[b, :, 0] = (ar < L).astype(f32) / (f32(L) * Nn)
        for b in a_idx:
            L = int(seq_len[b])
            k = L // 16 + 1
            sig = np.asarray(vaf_avf[b], np.float64)
            valid = ar < L
            o_s = np.argsort(np.where(valid, sig, np.inf), kind="stable")
            o_l = np.argsort(np.where(valid, -sig, np.inf), kind="stable")
            W[b, o_s[:k], 1] = 1.0 / (f32(k) * Na)
            W[b, o_l[:k], 2] = 1.0 / (f32(k) * Na)
    return W, Nn, Na


_runner_cache = {}


def _make_runner(nc):
    """Cached variant of bass2jax.run_bass_via_pjrt's multi-core path: jit
    once per program, reuse the compiled executable across kernel() calls."""
    import jax
    import numpy as _np
    from jax.experimental.shard_map import shard_map
    from jax.sharding import Mesh, PartitionSpec
    from concourse import bass2jax, mybir

    bass2jax.install_neuronx_cc_hook()
    assert nc.dbg_addr is None or not nc.dbg_callbacks
    partition_name = (nc.partition_id_tensor.name
                      if nc.partition_id_tensor else None)
    in_names, out_names, out_avals, zero_shapes = [], [], [], []
    for alloc in nc.m.functions[0].allocations:
        if not isinstance(alloc, mybir.MemoryLocationSet):
            continue
        name = alloc.memorylocations[0].name
        if alloc.kind == "ExternalInput":
            if name != partition_name:
                in_names.append(name)
        elif alloc.kind == "ExternalOutput":
            shape = tuple(alloc.tensor_shape)
            dtype = mybir.dt.np(alloc.dtype)
            out_names.append(name)
            out_avals.append(jax.core.ShapedArray(shape, dtype))
            zero_shapes.append((shape, dtype))
    n_params = len(in_names)
    n_outs = len(out_names)
    all_in = list(in_names) + list(out_names)
    if partition_name is not None:
        all_in.append(partition_name)
    donate = tuple(range(n_params, n_params + n_outs))

    def _body(*args):
        operands = list(args)
        if partition_name is not None:
            operands.append(bass2jax.partition_id_tensor())
        return tuple(bass2jax._bass_exec_p.bind(
            *operands,
            out_avals=tuple(out_avals),
            in_names=tuple(all_in),
            out_names=tuple(out_names),
            lowering_input_output_aliases=(),
            sim_require_finite=True,
            sim_require_nnan=True,
            nc=nc,
        ))

    devices = jax.devices()[:N_CORES]
    mesh = Mesh(_np.asarray(devices), ("core",))
    in_specs = (PartitionSpec("core"),) * (n_params + n_outs)
    out_specs = (PartitionSpec("core"),) * n_outs
    sharded = jax.jit(
        shard_map(_body, mesh=mesh, in_specs=in_specs, out_specs=out_specs,
                  check_rep=False),
        donate_argnums=donate, keep_unused=True)

    def run(in_maps):
        concat_in = [
            np.concatenate([np.asarray(m[name]) for m in in_maps], axis=0)
            for name in in_names
        ]
        concat_zeros = [
            np.zeros((N_CORES * s[0], *s[1:]), d) for (s, d) in zero_shapes
        ]
        out_arrs = sharded(*concat_in, *concat_zeros)
        return [
            {name: np.asarray(out_arrs[i]).reshape(
                N_CORES, *out_avals[i].shape)[c]
             for i, name in enumerate(out_names)}
            for c in range(N_CORES)
        ]

    return run


def _run_spmd(nc, in_maps):
    key = id(nc)
    if key not in _runner_cache:
        _runner_cache[key] = _make_runner(nc)
    return _runner_cache[key](in_maps)


def kernel(v_satt, va_satt, vf_satt, vaf_satt, v_avf, va_avf, vf_avf, vaf_avf,
           va_out, vf_out, vaf_out, lamda1, lamda2, lamda3, lamda4,
           label, seq_len):
    f32 = np.float32
    v = np.ascontiguousarray(np.asarray(v_satt, f32))
    oa = np.ascontiguousarray(np.asarray(va_satt, f32))
    of = np.ascontiguousarray(np.asarray(vf_satt, f32))
    vaf = np.ascontiguousarray(np.asarray(vaf_satt, f32))

    W, Nn, Na = _triplet_weights(label, seq_len, vaf_avf)

    if "p1" not in _prog_cache:
        import os
        _prog_cache["p1"] = _build_pass1(
            RPC, g_f32r=os.environ.get("G_F32R", "1") == "1")
    if "p2" not in _prog_cache:
        _prog_cache["p2"] = _build_pass2(RPC)

    def core_slice(x, c):
        return np.ascontiguousarray(
            x[c * SPC:(c + 1) * SPC].reshape(RPC, -1))

    in1 = [
        dict(v=core_slice(v, c), oa=core_slice(oa, c), of=core_slice(of, c),
             vaf=core_slice(vaf, c), w=core_slice(W, c))
        for c in range(N_CORES)
    ]
    res1 = _run_spmd(_prog_cache["p1"], in1)

    G_A = np.zeros((OM, M), np.float64)
    G_F = np.zeros((OM, M), np.float64)
    csq = np.zeros((3, M), np.float64)
    Tm = np.zeros((4, M), np.float64)
    for r in res1:
        G_A += r["ga"]
        G_F += r["gf"]
        csq += r["csq"]
        Tm += r["tm"]
    csqV = csq[0]
    csqA = csq[1, :OM]
    csqF = csq[1, OM:]

    nV = np.maximum(np.sqrt(csqV), 1e-12)
    simA = G_A / np.maximum(np.sqrt(csqA), 1e-12)[:, None] / nV[None, :]
    simF = G_F / np.maximum(np.sqrt(csqF), 1e-12)[:, None] / nV[None, :]
    extA = _greedy_ext(simA.astype(f32))
    extF = _greedy_ext(simF.astype(f32))

    def one_hot(ext):
        Pm = np.zeros((OM, M), ml_dtypes.bfloat16)
        j = np.arange(M)
        sel = ext < OM
        Pm[ext[sel], j[sel]] = 1.0
        return Pm

    in2 = [
        dict(v=core_slice(v, c), oa=core_slice(oa, c), of=core_slice(of, c),
             pa=one_hot(extA), pf=one_hot(extF))
        for c in range(N_CORES)
    ]
    res2 = _run_spmd(_prog_cache["p2"], in2)
    stats = np.concatenate([r["st"] for r in res2], axis=0)  # [B*T, 8]

    n1, n2, n3 = (stats[:, i].astype(np.float64) for i in range(3))
    rnV = np.sqrt(stats[:, 3].astype(np.float64) + stats[:, 4].astype(np.float64))
    rnA = np.sqrt(stats[:, 5].astype(np.float64))
    rnF = np.sqrt(stats[:, 6].astype(np.float64))

    def cos_term(num, rx, ry):
        den = np.maximum(rx * ry, 1e-8)
        return (1.0 - num / den).reshape(B, T).mean(1).sum()

    d_sum = (cos_term(n1, rnV, rnA) + cos_term(n2, rnV, rnF)
             + cos_term(n3, rnA, rnF)) / B

    ar = np.arange(T)
    seqm = (ar[None, :] < np.asarray(seq_len)[:, None]).astype(np.float64)
    Vs = np.asarray(v_avf, np.float64) * seqm
    As = np.asarray(va_avf, np.float64) * seqm
    Fs = np.asarray(vf_avf, np.float64) * seqm

    def ce(q, p):
        e = 1e-6
        q = np.clip(q, e, 1 - e)
        p = np.clip(p, e, 1 - e)
        return -(p * np.log(q) + (1 - p) * np.log(1 - q)).mean()

    ma_loss = d_sum + ce(Vs, As) + ce(Vs, Fs) + ce(As, Fs)

    yf = np.asarray(label).astype(np.float64)

    def bce(p, yy):
        p = np.asarray(p, np.float64)
        return -(yy * np.log(p) + (1 - yy) * np.log(1 - p)).mean()

    a_loss = bce(va_out, yf)
    f_loss = bce(vf_out, yf)
    raf_loss = bce(vaf_out, yf)

    if Nn == 0 or Na == 0:
        trip = 0.0
    else:
        anchor, pos, neg = Tm[0], Tm[1], Tm[2]
        nrm = lambda x: x / np.linalg.norm(x)
        a_, p_, g_ = nrm(anchor), nrm(pos), nrm(neg)
        d = lambda x, z: np.linalg.norm(x - z + 1e-6)
        trip = max(d(a_, p_) - d(a_, g_) + 5.0, 0.0)

    lam = [float(lamda1), float(lamda2), float(lamda3), float(lamda4)]
    total = (lam[0] * ma_loss + lam[1] * (a_loss + f_loss)
             + lam[2] * raf_loss + lam[3] * trip)
    return np.array([total, ma_loss, a_loss + f_loss, raf_loss, trip], f32)



# revision 30
# speedup vs baseline: 6.1336x; 6.1336x over previous
"""Trainium2 Bass kernel for the DISL loss (nn_DISL_Loss).

Strategy (data-parallel over batch rows, 8 cores, fp8 compute):
  Host: cast v/oa/of/vaf to fp8e4m3 (loss tolerance is 2e-2; measured
    end-to-end error of the full-fp8 pipeline is ~1e-4 because the greedy
    matching is degenerate and the loss aggregates over 16K rows).
  Pass 1 (device, per core): G_A = OA^T V and G_F = OF^T V partials via
    fp8 DoubleRow matmuls; column sums of squares of V via Act squares +
    PE ones-matmul (only V's column norms matter for the greedy argmax --
    row scaling of sim is argmax-invariant); per-row sums of squares of
    v/oa (Act square with fused accum_out) and of (DVE mult+reduce);
    triplet weighted row-sums Tm = W^T vaf (W prescaled x512 so fp8
    never underflows; the scale cancels under normalization).
  Host: all-reduce partials, sim = G / ||V_col||, greedy unique
    assignment (tiny, sequential), build one-hot gather matrices
    QA/QF [M,OM], Qg [OM,OM]: n1 = <OA_row, (V @ QA)_row>,
    n2 = <OF_row, (V @ QF)_row>, n3 = <OA_row, (OF @ Qg)_row>
    (exactly equivalent to the reference's padded-permutation cosine
    numerators; verified numerically).
  Pass 2 (device, transposed layout [feature, row] so the per-row
    reduction contracts over partitions on the PE): per (row-block,
    col-chunk): DoubleRow one-hot gather matmuls produce VgA^T/VgF^T/
    OFg^T in PSUM; DVE multiplies by host-pretransposed OA^T; PE
    ones-matmul accumulates the products into per-row stats n1/n2/n3.
  Host: cos/CE/BCE/triplet final combine (small tensors only).
"""

import numpy as np
import ml_dtypes

B, T, M, OM = 64, 256, 1024, 512
N_CORES = 8
SPC = B // N_CORES          # samples per core
RPC = SPC * T               # rows per core
P = 128
KT = RPC // P               # row-tiles per core
WSCALE = 512.0

F8 = ml_dtypes.float8_e4m3
BF = ml_dtypes.bfloat16

_prog_cache = {}


# ---------------------------------------------------------------- pass 1
def _build_pass1(rows):
    from concourse import bacc, mybir
    from concourse.tile import TileContext

    f32 = mybir.dt.float32
    fp8 = mybir.dt.float8e4
    DR = mybir.MatmulPerfMode.DoubleRow
    kt = rows // P
    npair = kt // 2

    nc = bacc.Bacc()
    v_d = nc.declare_dram_parameter("v8", [rows, M], fp8, isOutput=False)
    oa_d = nc.declare_dram_parameter("oa8", [rows, OM], fp8, isOutput=False)
    of_d = nc.declare_dram_parameter("of8", [rows, OM], fp8, isOutput=False)
    vaf_d = nc.declare_dram_parameter("vaf8", [rows, M], fp8, isOutput=False)
    # W is [rows, 4] logically; padded to 128 cols so the DoubleRow Tm
    # matmul has a full (128,128) ldweights tile (ISA check rejects col<64)
    w_d = nc.declare_dram_parameter("w8", [rows, P], fp8, isOutput=False)
    ga_d = nc.declare_dram_parameter("ga", [OM, M], fp8, isOutput=True)
    gf_d = nc.declare_dram_parameter("gf", [OM, M], fp8, isOutput=True)
    tm_d = nc.declare_dram_parameter("tm", [4, M], f32, isOutput=True)

    with TileContext(nc) as tc:
        with (
            tc.tile_pool(name="res", bufs=1) as rpool,
            tc.tile_pool(name="out", bufs=2) as opool,
        ):
            v_sb = rpool.tile([P, kt, M], fp8, tag="v")
            oa_sb = rpool.tile([P, kt, OM], fp8, tag="oa")
            of_sb = rpool.tile([P, kt, OM], fp8, tag="of")
            vaf_sb = rpool.tile([P, kt, M], fp8, tag="vaf")
            w_sb = rpool.tile([P, kt, P], fp8, tag="w")

            # loads spread over four DMA queues (they transfer concurrently)
            def ld1(dram, sb, i, chunks, eng):
                per = kt // chunks
                eng.dma_start(
                    sb[:, i * per:(i + 1) * per, :],
                    dram[i * per * P:(i + 1) * per * P, :].rearrange(
                        "(k p) m -> p k m", p=P))

            ld1(oa_d, oa_sb, 0, 2, nc.sync)
            ld1(v_d, v_sb, 0, 4, nc.gpsimd)
            ld1(v_d, v_sb, 1, 4, nc.scalar)
            ld1(oa_d, oa_sb, 1, 2, nc.sync)
            ld1(v_d, v_sb, 2, 4, nc.gpsimd)
            ld1(v_d, v_sb, 3, 4, nc.scalar)
            ld1(of_d, of_sb, 0, 2, nc.sync)
            ld1(of_d, of_sb, 1, 2, nc.gpsimd)
            ld1(vaf_d, vaf_sb, 0, 2, nc.scalar)
            ld1(vaf_d, vaf_sb, 1, 2, nc.sync)
            nc.gpsimd.dma_start(
                w_sb[:], w_d.rearrange("(k p) c -> p k c", p=P))

            gab = rpool.tile([P, 4, M], fp8, tag="gab")
            gfb = rpool.tile([P, 4, M], fp8, tag="gfb")

            # five "quarters" rotating through one PSUM pool (bufs=2):
            # G_A mc01, G_A mc23, Tm, G_F mc01, G_F mc23.  Each quarter's
            # psum->sbuf copies and output DMA overlap the next quarter.
            with tc.tile_pool(name="psq", bufs=2, space="PSUM") as psq:
                def g_quarter(src_sb, mch, gsb, g_dram, tagc):
                    tiles = []
                    for nh in range(2):
                        gp = psq.tile([P, 2, 512], f32, tag=f"q{nh}",
                                      name=f"g{tagc}{mch}_{nh}")
                        tiles.append(gp)
                    for jp in range(npair):
                        for mi in range(2):
                            mc = mch * 2 + mi
                            for nh in range(2):
                                nc.tensor.matmul(
                                    tiles[nh][:, mi, :],
                                    lhsT=src_sb[:, 2 * jp:2 * jp + 2,
                                                mc * P:(mc + 1) * P],
                                    rhs=v_sb[:, 2 * jp:2 * jp + 2,
                                             nh * 512:(nh + 1) * 512],
                                    start=(jp == 0), stop=(jp == npair - 1),
                                    perf_mode=DR)
                    lo = mch * 2
                    nc.scalar.copy(gsb[:, lo:lo + 2, 0:512], tiles[0][:])
                    nc.vector.tensor_copy(gsb[:, lo:lo + 2, 512:M],
                                          tiles[1][:])
                    eng = nc.sync if mch == 0 else nc.gpsimd
                    eng.dma_start(
                        g_dram[mch * 256:(mch + 1) * 256, :].rearrange(
                            "(c p) m -> p c m", p=P),
                        gsb[:, lo:lo + 2, :])

                g_quarter(oa_sb, 0, gab, ga_d, "a")
                g_quarter(oa_sb, 1, gab, ga_d, "a")

                # Tm quarter: both nh halves in one [P, 2, 512] tile
                tmps = psq.tile([P, 2, 512], f32, tag="q0", name="tmq")
                for jp in range(npair):
                    for nh in range(2):
                        nc.tensor.matmul(
                            tmps[:, nh, :],
                            lhsT=w_sb[:, 2 * jp:2 * jp + 2, :],
                            rhs=vaf_sb[:, 2 * jp:2 * jp + 2,
                                       nh * 512:(nh + 1) * 512],
                            start=(jp == 0), stop=(jp == npair - 1),
                            perf_mode=DR)
                tmo = opool.tile([4, 2, 512], f32, tag="tmo")
                nc.scalar.copy(tmo[:], tmps[0:4, :, :])
                nc.sync.dma_start(
                    tm_d.rearrange("a (b m) -> a b m", b=2), tmo[:])

                g_quarter(of_sb, 0, gfb, gf_d, "f")
                g_quarter(of_sb, 1, gfb, gf_d, "f")
    nc.finalize()
    return nc


# ---------------------------------------------------------------- pass 2
def _build_pass2(rows):
    from concourse import bacc, mybir
    from concourse.tile import TileContext

    f32 = mybir.dt.float32
    bf16 = mybir.dt.bfloat16
    fp8 = mybir.dt.float8e4
    DR = mybir.MatmulPerfMode.DoubleRow
    RB = rows // 512            # 512-row blocks

    nc = bacc.Bacc()
    vt_d = nc.declare_dram_parameter("vt8", [M, rows], fp8, isOutput=False)
    oat_d = nc.declare_dram_parameter("oatb", [OM, rows], bf16,
                                      isOutput=False)
    oft_d = nc.declare_dram_parameter("oft8", [OM, rows], fp8, isOutput=False)
    qa_d = nc.declare_dram_parameter("qa8", [M, OM], fp8, isOutput=False)
    qf_d = nc.declare_dram_parameter("qf8", [M, OM], fp8, isOutput=False)
    qg_d = nc.declare_dram_parameter("qg8", [OM, OM], fp8, isOutput=False)
    nst_d = nc.declare_dram_parameter("nst", [rows // 512, 3, 512], f32,
                                      isOutput=True)

    with TileContext(nc) as tc:
        with (
            tc.tile_pool(name="res", bufs=1) as rpool,
            tc.tile_pool(name="scr", bufs=4) as spool,
        ):
            vt_sb = rpool.tile([P, 8, rows], fp8, tag="vt")
            oat_sb = rpool.tile([P, 4, rows], bf16, tag="oat")
            oft_sb = rpool.tile([P, 4, rows], fp8, tag="oft")
            qa_sb = rpool.tile([P, 8, OM], fp8, tag="qa")
            qf_sb = rpool.tile([P, 8, OM], fp8, tag="qf")
            qg_sb = rpool.tile([P, 4, OM], fp8, tag="qg")
            onesb = rpool.tile([P, 1], bf16, tag="onesb")
            # fp8 DoubleRow ones-selectors: colmask[:, :, s, c] = (c == s).
            # A DR ones-matmul with lhsT = colmask[:, :, s, :] writes the
            # partition-sum of rhs into psum ROW s (other rows += 0), so
            # n2 and n3 share a single psum bank / accumulation group.
            colmask = rpool.tile([P, 2, 2, P], fp8, tag="colmask")

            nc.vector.memset(onesb[:], 1.0)
            nc.vector.memset(colmask[:], 0.0)
            for s in range(2):
                nc.vector.memset(colmask[:, :, s, s:s + 1], 1.0)

            # row-block-major loads so row-block 0 compute starts early
            nc.sync.dma_start(
                qa_sb[:], qa_d.rearrange("(c p) m -> p c m", p=P))

            def ld_blk(dram, sb, nch, rb, eng):
                r0 = rb * 512
                eng.dma_start(
                    sb[:, :, r0:r0 + 512],
                    dram[:, r0:r0 + 512].rearrange("(c p) r -> p c r", p=P))

            ld_blk(vt_d, vt_sb, 8, 0, nc.gpsimd)
            ld_blk(oat_d, oat_sb, 4, 0, nc.sync)
            ld_blk(oft_d, oft_sb, 4, 0, nc.sync)
            nc.sync.dma_start(
                qf_sb[:], qf_d.rearrange("(c p) m -> p c m", p=P))
            nc.gpsimd.dma_start(
                qg_sb[:], qg_d.rearrange("(c p) m -> p c m", p=P))
            engs = [nc.sync, nc.gpsimd]
            for rb in range(1, RB):
                ld_blk(vt_d, vt_sb, 8, rb, engs[rb % 2])
                ld_blk(oat_d, oat_sb, 4, rb, engs[(rb + 1) % 2])
                ld_blk(oft_d, oft_sb, 4, rb, engs[rb % 2])

            # stats: transposed layout; DoubleRow gathers fill PSUM; n1 goes
            # through an Act bf16 copy for a 2x sbuf DVE multiply; n2/n3 are
            # direct 1x DVE multiplies from PSUM; per-row reduction contracts
            # partitions on the PE (ones-matmuls).
            with (
                tc.tile_pool(name="pga", bufs=1, space="PSUM") as pga,
                tc.tile_pool(name="pgf", bufs=1, space="PSUM") as pgf,
                tc.tile_pool(name="pgo", bufs=1, space="PSUM") as pgo,
                tc.tile_pool(name="ps1", bufs=1, space="PSUM") as ps1,
                tc.tile_pool(name="ps23", bufs=1, space="PSUM") as ps23,
            ):
                for rb in range(RB):
                    r0 = rb * 512
                    stat1 = ps1.tile([1, 512], f32, tag="s1",
                                     name=f"s1_{rb}")
                    stat23 = ps23.tile([P, 512], f32, tag="s23",
                                       name=f"s23_{rb}")
                    first23 = [True]
                    for ccp in range(2):
                        # --- n2: VgF pair, direct 1x DVE mult, DR ones row 0
                        gf_ = pgf.tile([P, 2, 512], f32, tag="gf",
                                       name=f"gf{rb}_{ccp}")
                        for s in range(2):
                            cc = 2 * ccp + s
                            for i in range(4):
                                nc.tensor.matmul(
                                    gf_[:, s, :],
                                    lhsT=qf_sb[:, 2 * i:2 * i + 2,
                                               cc * P:(cc + 1) * P],
                                    rhs=vt_sb[:, 2 * i:2 * i + 2,
                                              r0:r0 + 512],
                                    start=(i == 0), stop=(i == 3),
                                    perf_mode=DR)
                        pr2 = spool.tile([P, 2, 512], fp8, tag="pr2",
                                         name=f"pr2{rb}_{ccp}")
                        nc.vector.tensor_mul(
                            pr2[:], oft_sb[:, 2 * ccp:2 * ccp + 2,
                                           r0:r0 + 512], gf_[:])
                        nc.tensor.matmul(
                            stat23[:], lhsT=colmask[:, :, 0, :], rhs=pr2[:],
                            start=first23[0], stop=False, perf_mode=DR)
                        first23[0] = False
                        # --- n3: OFg pair, direct 1x DVE mult, DR ones row 1
                        og = pgo.tile([P, 2, 512], f32, tag="og",
                                      name=f"og{rb}_{ccp}")
                        for s in range(2):
                            cc = 2 * ccp + s
                            for i in range(2):
                                nc.tensor.matmul(
                                    og[:, s, :],
                                    lhsT=qg_sb[:, 2 * i:2 * i + 2,
                                               cc * P:(cc + 1) * P],
                                    rhs=oft_sb[:, 2 * i:2 * i + 2,
                                               r0:r0 + 512],
                                    start=(i == 0), stop=(i == 1),
                                    perf_mode=DR)
                        pr3 = spool.tile([P, 2, 512], fp8, tag="pr3",
                                         name=f"pr3{rb}_{ccp}")
                        nc.vector.tensor_mul(
                            pr3[:], oat_sb[:, 2 * ccp:2 * ccp + 2,
                                           r0:r0 + 512], og[:])
                        nc.tensor.matmul(
                            stat23[:], lhsT=colmask[:, :, 1, :], rhs=pr3[:],
                            start=False, stop=(ccp == 1), perf_mode=DR)
                        # --- n1: VgA pair -> Act copy -> 2x DVE mult
                        ga = pga.tile([P, 2, 512], f32, tag="ga",
                                      name=f"ga{rb}_{ccp}")
                        for s in range(2):
                            cc = 2 * ccp + s
                            for i in range(4):
                                nc.tensor.matmul(
                                    ga[:, s, :],
                                    lhsT=qa_sb[:, 2 * i:2 * i + 2,
                                               cc * P:(cc + 1) * P],
                                    rhs=vt_sb[:, 2 * i:2 * i + 2,
                                              r0:r0 + 512],
                                    start=(i == 0), stop=(i == 3),
                                    perf_mode=DR)
                        cA = spool.tile([P, 2, 512], bf16, tag="cA",
                                        name=f"cA{rb}_{ccp}")
                        nc.scalar.copy(cA[:], ga[:])
                        pr1 = spool.tile([P, 2, 512], bf16, tag="pr1",
                                         name=f"pr1{rb}_{ccp}")
                        nc.vector.tensor_mul(
                            pr1[:], oat_sb[:, 2 * ccp:2 * ccp + 2,
                                           r0:r0 + 512], cA[:])
                        for s in range(2):
                            nc.tensor.matmul(
                                stat1[:], lhsT=onesb[:], rhs=pr1[:, s, :],
                                start=(ccp == 0 and s == 0),
                                stop=(ccp == 1 and s == 1))
                    n1so = spool.tile([1, 512], f32, tag="n1so",
                                      name=f"n1so{rb}")
                    n23so = spool.tile([2, 512], f32, tag="n23so",
                                       name=f"n23so{rb}")
                    nc.scalar.copy(n1so[:], stat1[:])
                    nc.scalar.copy(n23so[:], stat23[0:2, :])
                    nc.sync.dma_start(nst_d[rb, 0:1, :], n1so[:])
                    nc.gpsimd.dma_start(nst_d[rb, 1:3, :], n23so[:])
    nc.finalize()
    return nc


# ---------------------------------------------------------------- host math
def _greedy_ext(sim):
    om, m = sim.shape
    used = np.zeros(m, dtype=bool)
    I = np.empty(om, dtype=np.int32)
    for r in range(om):
        row = np.where(used, -np.inf, sim[r])
        c = int(np.argmax(row))
        I[r] = c
        used[c] = True
    ext = np.empty(m, dtype=np.int32)
    ext[:om] = I
    ext[om:] = np.nonzero(~used)[0]
    return ext


def _triplet_weights(label, seq_len, vaf_avf):
    f32 = np.float32
    y = np.asarray(label).astype(np.int64)
    n_idx = np.nonzero(y == 0)[0]
    a_idx = np.nonzero(y == 1)[0]
    W = np.zeros((B, T, 4), f32)
    ar = np.arange(T)
    Nn, Na = len(n_idx), len(a_idx)
    if Nn and Na:
        for b in n_idx:
            L = int(seq_len[b])
            W[b, :, 0] = (ar < L).astype(f32) * WSCALE / (f32(L) * Nn)
        for b in a_idx:
            L = int(seq_len[b])
            k = L // 16 + 1
            sig = np.asarray(vaf_avf[b], np.float64)
            valid = ar < L
            o_s = np.argsort(np.where(valid, sig, np.inf), kind="stable")
            o_l = np.argsort(np.where(valid, -sig, np.inf), kind="stable")
            W[b, o_s[:k], 1] = WSCALE / (f32(k) * Na)
            W[b, o_l[:k], 2] = WSCALE / (f32(k) * Na)
    return W, Nn, Na


_runner_cache = {}


def _make_runner(nc):
    """Cached variant of bass2jax.run_bass_via_pjrt's multi-core path: jit
    once per program, reuse the compiled executable across kernel() calls."""
    import jax
    import numpy as _np
    from jax.experimental.shard_map import shard_map
    from jax.sharding import Mesh, PartitionSpec
    from concourse import bass2jax, mybir

    bass2jax.install_neuronx_cc_hook()
    assert nc.dbg_addr is None or not nc.dbg_callbacks
    partition_name = (nc.partition_id_tensor.name
                      if nc.partition_id_tensor else None)
    in_names, out_names, out_avals, zero_shapes = [], [], [], []
    for alloc in nc.m.functions[0].allocations:
        if not isinstance(alloc, mybir.MemoryLocationSet):
            continue
        name = alloc.memorylocations[0].name
        if alloc.kind == "ExternalInput":
            if name != partition_name:
                in_names.append(name)
        elif alloc.kind == "ExternalOutput":
            shape = tuple(alloc.tensor_shape)
            dtype = mybir.dt.np(alloc.dtype)
            out_names.append(name)
            out_avals.append(jax.core.ShapedArray(shape, dtype))
            zero_shapes.append((shape, dtype))
    n_params = len(in_names)
    n_outs = len(out_names)
    all_in = list(in_names) + list(out_names)
    if partition_name is not None:
        all_in.append(partition_name)
    donate = tuple(range(n_params, n_params + n_outs))

    def _body(*args):
        operands = list(args)
        if partition_name is not None:
            operands.append(bass2jax.partition_id_tensor())
        return tuple(bass2jax._bass_exec_p.bind(
            *operands,
            out_avals=tuple(out_avals),
            in_names=tuple(all_in),
            out_names=tuple(out_names),
            lowering_input_output_aliases=(),
            sim_require_finite=True,
            sim_require_nnan=True,
            nc=nc,
        ))

    devices = jax.devices()[:N_CORES]
    mesh = Mesh(_np.asarray(devices), ("core",))
    in_specs = (PartitionSpec("core"),) * (n_params + n_outs)
    out_specs = (PartitionSpec("core"),) * n_outs
    sharded = jax.jit(
        shard_map(_body, mesh=mesh, in_specs=in_specs, out_specs=out_specs,
                  check_rep=False),
        donate_argnums=donate, keep_unused=True)

    def run(in_maps):
        concat_in = [
            np.concatenate([np.asarray(m[name]) for m in in_maps], axis=0)
            for name in in_names
        ]
        concat_zeros = [
            np.zeros((N_CORES * s[0], *s[1:]), d) for (s, d) in zero_shapes
        ]
        out_arrs = sharded(*concat_in, *concat_zeros)
        return [
            {name: np.asarray(out_arrs[i]).reshape(
                N_CORES, *out_avals[i].shape)[c]
             for i, name in enumerate(out_names)}
            for c in range(N_CORES)
        ]

    return run


def _run_spmd(nc, in_maps):
    key = id(nc)
    if key not in _runner_cache:
        _runner_cache[key] = _make_runner(nc)
    return _runner_cache[key](in_maps)


def kernel(v_satt, va_satt, vf_satt, vaf_satt, v_avf, va_avf, vf_avf, vaf_avf,
           va_out, vf_out, vaf_out, lamda1, lamda2, lamda3, lamda4,
           label, seq_len):
    f32 = np.float32
    v8 = np.asarray(v_satt, f32).reshape(B * T, M).astype(F8)
    oa8 = np.asarray(va_satt, f32).reshape(B * T, OM).astype(F8)
    of8 = np.asarray(vf_satt, f32).reshape(B * T, OM).astype(F8)
    vaf8 = np.asarray(vaf_satt, f32).reshape(B * T, M).astype(F8)

    W, Nn, Na = _triplet_weights(label, seq_len, vaf_avf)
    w8 = np.zeros((B * T, P), F8)
    w8[:, 0:4] = W.reshape(B * T, 4).astype(F8)

    if "p1" not in _prog_cache:
        _prog_cache["p1"] = _build_pass1(RPC)
    if "p2" not in _prog_cache:
        _prog_cache["p2"] = _build_pass2(RPC)

    def sl(x, c):
        return x[c * RPC:(c + 1) * RPC]

    in1 = [
        dict(v8=sl(v8, c), oa8=sl(oa8, c), of8=sl(of8, c),
             vaf8=sl(vaf8, c), w8=sl(w8, c))
        for c in range(N_CORES)
    ]
    res1 = _run_spmd(_prog_cache["p1"], in1)

    G_A = np.zeros((OM, M), np.float64)
    G_F = np.zeros((OM, M), np.float64)
    Tm = np.zeros((4, M), np.float64)
    for r in res1:
        G_A += r["ga"].astype(np.float64)
        G_F += r["gf"].astype(np.float64)
        Tm += r["tm"]

    # norms: cheap O(n) scalar summaries, computed host-side from the same
    # fp8-rounded values the device consumes
    v8f = v8.astype(f32)
    oa8f = oa8.astype(f32)
    of8f = of8.astype(f32)
    sqV = np.square(v8f)
    nV = np.maximum(np.sqrt(sqV.sum(0)), 1e-12)
    rnV = np.sqrt(sqV.sum(1, dtype=np.float64))
    rnA = np.sqrt(np.square(oa8f).sum(1, dtype=np.float64))
    rnF = np.sqrt(np.square(of8f).sum(1, dtype=np.float64))
    extA = _greedy_ext((G_A / nV[None, :]).astype(f32))
    extF = _greedy_ext((G_F / nV[None, :]).astype(f32))

    # gather matrices: VgA[:, c] = V[:, invA[c]];  OFg[:, c] = OF[:, g[c]]
    invA = np.empty(M, np.int64)
    invA[extA] = np.arange(M)
    invF = np.empty(M, np.int64)
    invF[extF] = np.arange(M)
    QA = np.zeros((M, OM), F8)
    QA[invA[:OM], np.arange(OM)] = 1.0
    QF = np.zeros((M, OM), F8)
    QF[invF[:OM], np.arange(OM)] = 1.0
    g = extF[invA[:OM]]
    Qg = np.zeros((OM, OM), F8)
    selg = g < OM
    Qg[g[selg], np.arange(OM)[selg]] = 1.0

    vt8 = np.ascontiguousarray(
        v8.reshape(N_CORES, RPC, M).transpose(0, 2, 1))
    oatb = np.ascontiguousarray(
        oa8.astype(BF).reshape(N_CORES, RPC, OM).transpose(0, 2, 1))
    oft8 = np.ascontiguousarray(
        of8.reshape(N_CORES, RPC, OM).transpose(0, 2, 1))

    in2 = [
        dict(vt8=vt8[c], oatb=oatb[c], oft8=oft8[c],
             qa8=QA, qf8=QF, qg8=Qg)
        for c in range(N_CORES)
    ]
    res2 = _run_spmd(_prog_cache["p2"], in2)
    nst = np.concatenate(
        [r["nst"].transpose(1, 0, 2).reshape(3, RPC) for r in res2],
        axis=1)  # [3, B*T]

    n1 = nst[0].astype(np.float64)
    n2 = nst[1].astype(np.float64)
    n3 = nst[2].astype(np.float64)

    def cos_term(num, rx, ry):
        den = np.maximum(rx * ry, 1e-8)
        return (1.0 - num / den).reshape(B, T).mean(1).sum()

    d_sum = (cos_term(n1, rnV, rnA) + cos_term(n2, rnV, rnF)
             + cos_term(n3, rnA, rnF)) / B

    ar = np.arange(T)
    seqm = (ar[None, :] < np.asarray(seq_len)[:, None]).astype(np.float64)
    Vs = np.asarray(v_avf, np.float64) * seqm
    As = np.asarray(va_avf, np.float64) * seqm
    Fs = np.asarray(vf_avf, np.float64) * seqm

    def ce(q, p):
        e = 1e-6
        q = np.clip(q, e, 1 - e)
        p = np.clip(p, e, 1 - e)
        return -(p * np.log(q) + (1 - p) * np.log(1 - q)).mean()

    ma_loss = d_sum + ce(Vs, As) + ce(Vs, Fs) + ce(As, Fs)

    yf = np.asarray(label).astype(np.float64)

    def bce(p, yy):
        p = np.asarray(p, np.float64)
        return -(yy * np.log(p) + (1 - yy) * np.log(1 - p)).mean()

    a_loss = bce(va_out, yf)
    f_loss = bce(vf_out, yf)
    raf_loss = bce(vaf_out, yf)

    if Nn == 0 or Na == 0:
        trip = 0.0
    else:
        anchor, pos, neg = Tm[0] / WSCALE, Tm[1] / WSCALE, Tm[2] / WSCALE
        nrm = lambda x: x / np.linalg.norm(x)
        a_, p_, g_ = nrm(anchor), nrm(pos), nrm(neg)
        d = lambda x, z: np.linalg.norm(x - z + 1e-6)
        trip = max(d(a_, p_) - d(a_, g_) + 5.0, 0.0)

    lam = [float(lamda1), float(lamda2), float(lamda3), float(lamda4)]
    total = (lam[0] * ma_loss + lam[1] * (a_loss + f_loss)
             + lam[2] * raf_loss + lam[3] * trip)
    return np.array([total, ma_loss, a_loss + f_loss, raf_loss, trip], f32)


# revision 48
# speedup vs baseline: 6.6967x; 1.0918x over previous
"""Trainium2 Bass kernel for the DISL loss (nn_DISL_Loss).

Strategy (data-parallel over batch rows, 8 cores, fp8/bf16 compute):
  Host: cast v/oa/of/vaf to fp8e4m3 (loss tolerance is 2e-2; measured
    end-to-end error of the low-precision pipeline is ~2e-4 because the
    greedy matching is degenerate and the loss aggregates over 16K rows).
  Pass 1 (device, per core): G_A = OA^T V and G_F = OF^T V partials via
    fp8 DoubleRow matmuls (4x bf16 rate in the cost model), plus the
    triplet row-sums Tm = W^T vaf (W prescaled x512 so fp8 never
    underflows; the scale cancels under normalization). Five PSUM
    "quarters" rotate through one pool so copies/output DMA overlap the
    next quarter; loads are spread over the three DMA queues (they
    transfer concurrently).
  Host: all-reduce the G partials, sim = G / ||V_col|| (only V's column
    norms matter: row scaling of sim is argmax-invariant), greedy unique
    assignment (tiny, sequential), inverse-permutation index vectors.
    Row norms rnV/rnA/rnF are cheap O(n) host reductions.
    n1 = <OA_row, (V @ QA)_row>, n2 = <OF_row, (V @ QF)_row>,
    n3 = <OA_row, (OF @ Qg)_row> -- exactly equivalent to the reference's
    padded-permutation cosine numerators (verified numerically).
  Pass 2 (device, transposed [feature, row] layout): VgA^T/VgF^T come
    from dma_gather (SWDGE row-gathers straight from HBM by runtime
    int16 indices, prepared on two queues + triggered, consumers wait on
    the completion semaphores); OFg^T via PE DoubleRow one-hot matmuls
    (overlapping the gathers). Products are 2x DVE multiplies on SBUF
    bf16; the per-row reduction contracts partitions on PE ones-matmuls.
  Host: cos/CE/BCE/triplet final combine (small tensors only).
"""

import numpy as np
import ml_dtypes

B, T, M, OM = 64, 256, 1024, 512
N_CORES = 8
SPC = B // N_CORES          # samples per core
RPC = SPC * T               # rows per core
P = 128
KT = RPC // P               # row-tiles per core
WSCALE = 512.0

F8 = ml_dtypes.float8_e4m3
BF = ml_dtypes.bfloat16

_prog_cache = {}


# ---------------------------------------------------------------- pass 1
def _build_pass1(rows):
    from concourse import bacc, mybir
    from concourse.tile import TileContext

    f32 = mybir.dt.float32
    fp8 = mybir.dt.float8e4
    DR = mybir.MatmulPerfMode.DoubleRow
    kt = rows // P
    npair = kt // 2

    nc = bacc.Bacc()
    v_d = nc.declare_dram_parameter("v8", [rows, M], fp8, isOutput=False)
    oa_d = nc.declare_dram_parameter("oa8", [rows, OM], fp8, isOutput=False)
    of_d = nc.declare_dram_parameter("of8", [rows, OM], fp8, isOutput=False)
    vaf_d = nc.declare_dram_parameter("vaf8", [rows, M], fp8, isOutput=False)
    # W is [rows, 4] logically; padded to 128 cols so the DoubleRow Tm
    # matmul has a full (128,128) ldweights tile (ISA check rejects col<64)
    w_d = nc.declare_dram_parameter("w8", [rows, P], fp8, isOutput=False)
    ga_d = nc.declare_dram_parameter("ga", [OM, M], fp8, isOutput=True)
    gf_d = nc.declare_dram_parameter("gf", [OM, M], fp8, isOutput=True)
    tm_d = nc.declare_dram_parameter("tm", [4, M], f32, isOutput=True)

    with TileContext(nc) as tc:
        with (
            tc.tile_pool(name="res", bufs=1) as rpool,
            tc.tile_pool(name="out", bufs=2) as opool,
        ):
            v_sb = rpool.tile([P, kt, M], fp8, tag="v")
            oa_sb = rpool.tile([P, kt, OM], fp8, tag="oa")
            of_sb = rpool.tile([P, kt, OM], fp8, tag="of")
            vaf_sb = rpool.tile([P, kt, M], fp8, tag="vaf")
            w_sb = rpool.tile([P, kt, P], fp8, tag="w")

            # loads spread over four DMA queues (they transfer concurrently)
            def ld1(dram, sb, i, chunks, eng):
                per = kt // chunks
                eng.dma_start(
                    sb[:, i * per:(i + 1) * per, :],
                    dram[i * per * P:(i + 1) * per * P, :].rearrange(
                        "(k p) m -> p k m", p=P))

            def ldr(dram, sb, k0, k1, eng):
                eng.dma_start(
                    sb[:, k0:k1, :],
                    dram[k0 * P:k1 * P, :].rearrange(
                        "(k p) m -> p k m", p=P))

            ldr(oa_d, oa_sb, 0, 2, nc.sync)
            ldr(v_d, v_sb, 0, 2, nc.gpsimd)
            ldr(v_d, v_sb, 2, 6, nc.scalar)
            ldr(oa_d, oa_sb, 2, 9, nc.sync)
            ldr(v_d, v_sb, 6, 11, nc.gpsimd)
            ldr(v_d, v_sb, 11, 16, nc.scalar)
            ldr(oa_d, oa_sb, 9, 16, nc.sync)
            ldr(of_d, of_sb, 0, 8, nc.gpsimd)
            ldr(of_d, of_sb, 8, 16, nc.sync)
            ldr(vaf_d, vaf_sb, 0, 8, nc.scalar)
            ldr(vaf_d, vaf_sb, 8, 16, nc.sync)
            nc.gpsimd.dma_start(
                w_sb[:], w_d.rearrange("(k p) c -> p k c", p=P))

            gab = rpool.tile([P, 4, M], fp8, tag="gab")
            gfb = rpool.tile([P, 4, M], fp8, tag="gfb")

            # five "quarters" rotating through one PSUM pool (bufs=2):
            # G_A mc01, G_A mc23, Tm, G_F mc01, G_F mc23.  Each quarter's
            # psum->sbuf copies and output DMA overlap the next quarter.
            with tc.tile_pool(name="psq", bufs=2, space="PSUM") as psq:
                def g_quarter(src_sb, mch, gsb, g_dram, tagc):
                    tiles = []
                    for nh in range(2):
                        gp = psq.tile([P, 2, 512], f32, tag=f"q{nh}",
                                      name=f"g{tagc}{mch}_{nh}")
                        tiles.append(gp)
                    for jp in range(npair):
                        for mi in range(2):
                            mc = mch * 2 + mi
                            for nh in range(2):
                                nc.tensor.matmul(
                                    tiles[nh][:, mi, :],
                                    lhsT=src_sb[:, 2 * jp:2 * jp + 2,
                                                mc * P:(mc + 1) * P],
                                    rhs=v_sb[:, 2 * jp:2 * jp + 2,
                                             nh * 512:(nh + 1) * 512],
                                    start=(jp == 0), stop=(jp == npair - 1),
                                    perf_mode=DR)
                    lo = mch * 2
                    nc.scalar.copy(gsb[:, lo:lo + 2, 0:512], tiles[0][:])
                    nc.vector.tensor_copy(gsb[:, lo:lo + 2, 512:M],
                                          tiles[1][:])
                    eng = nc.sync if mch == 0 else nc.gpsimd
                    eng.dma_start(
                        g_dram[mch * 256:(mch + 1) * 256, :].rearrange(
                            "(c p) m -> p c m", p=P),
                        gsb[:, lo:lo + 2, :])

                g_quarter(oa_sb, 0, gab, ga_d, "a")
                g_quarter(oa_sb, 1, gab, ga_d, "a")

                # Tm quarter: both nh halves in one [P, 2, 512] tile
                tmps = psq.tile([P, 2, 512], f32, tag="q0", name="tmq")
                for jp in range(npair):
                    for nh in range(2):
                        nc.tensor.matmul(
                            tmps[:, nh, :],
                            lhsT=w_sb[:, 2 * jp:2 * jp + 2, :],
                            rhs=vaf_sb[:, 2 * jp:2 * jp + 2,
                                       nh * 512:(nh + 1) * 512],
                            start=(jp == 0), stop=(jp == npair - 1),
                            perf_mode=DR)
                tmo = opool.tile([4, 2, 512], f32, tag="tmo")
                nc.scalar.copy(tmo[:], tmps[0:4, :, :])
                nc.sync.dma_start(
                    tm_d.rearrange("a (b m) -> a b m", b=2), tmo[:])

                g_quarter(of_sb, 0, gfb, gf_d, "f")
                g_quarter(of_sb, 1, gfb, gf_d, "f")
    nc.finalize()
    return nc


# ---------------------------------------------------------------- pass 2
def _build_pass2(rows):
    from concourse import bacc, mybir
    from concourse.tile import TileContext
    from concourse.library_config import mlp

    f32 = mybir.dt.float32
    bf16 = mybir.dt.bfloat16
    fp8 = mybir.dt.float8e4
    i16 = mybir.dt.int16
    DR = mybir.MatmulPerfMode.DoubleRow
    RB = rows // 512            # 512-row stat blocks

    nc = bacc.Bacc(num_swdge_queues=2)
    # gather source stays in HBM (never DMA'd whole)
    vtb_d = nc.declare_dram_parameter("vtb", [M, rows], bf16, isOutput=False)
    oatb_d = nc.declare_dram_parameter("oatb", [OM, rows], bf16,
                                       isOutput=False)
    oftb_d = nc.declare_dram_parameter("oftb", [OM, rows], bf16,
                                       isOutput=False)
    oft8_d = nc.declare_dram_parameter("oft8", [OM, rows], fp8,
                                       isOutput=False)
    qg_d = nc.declare_dram_parameter("qg8", [OM, OM], fp8, isOutput=False)
    ixa_d = nc.declare_dram_parameter("ixa", [P, OM // 16], i16,
                                      isOutput=False)
    ixf_d = nc.declare_dram_parameter("ixf", [P, OM // 16], i16,
                                      isOutput=False)
    nst_d = nc.declare_dram_parameter("nst", [3, RB, 512], f32,
                                      isOutput=True)

    with TileContext(nc) as tc:
        with (
            tc.tile_pool(name="res", bufs=1) as rpool,
            tc.tile_pool(name="scr", bufs=3) as spool,
        ):
            oat_sb = rpool.tile([P, 4, rows], bf16, tag="oat")
            oftb_sb = rpool.tile([P, 4, rows], bf16, tag="oftb")
            oft8_sb = rpool.tile([P, 4, rows], fp8, tag="oft8")
            qg_sb = rpool.tile([P, 4, OM], fp8, tag="qg")
            ixa = rpool.tile([P, OM // 16], i16, tag="ixa")
            ixf = rpool.tile([P, OM // 16], i16, tag="ixf")
            ga_sb = rpool.tile([P, 4, rows], bf16, tag="ga")
            gf_sb = rpool.tile([P, 4, rows], bf16, tag="gf")
            onesb = rpool.tile([P, 1], bf16, tag="onesb")
            nc.vector.memset(onesb[:], 1.0)

            nc.gpsimd.load_library(mlp)
            nc.gpsimd.dma_start(ixa[:], ixa_d[:, :])
            nc.gpsimd.dma_start(ixf[:], ixf_d[:, :])
            gsems = [nc.alloc_semaphore(f"gsem{q}") for q in range(2)]
            nc.gpsimd.dma_gather(ga_sb[:], vtb_d[:, :], ixa[:], OM, OM,
                                 rows, prepare_only=True, sem=gsems[0],
                                 queue_num=0)
            nc.gpsimd.dma_gather(gf_sb[:], vtb_d[:, :], ixf[:], OM, OM,
                                 rows, prepare_only=True, sem=gsems[1],
                                 queue_num=1)
            nc.gpsimd.trigger_dma(count=None, queue_num=0)
            nc.gpsimd.trigger_dma(count=None, queue_num=1)
            # n3 operands load first (its compute overlaps the gathers)
            nc.sync.dma_start(
                qg_sb[:], qg_d.rearrange("(c p) m -> p c m", p=P))
            for i in range(2):
                nc.sync.dma_start(
                    oft8_sb[:, i * 2:(i + 1) * 2, :],
                    oft8_d[i * 2 * P:(i + 1) * 2 * P, :].rearrange(
                        "(c p) r -> p c r", p=P))
            for i in range(2):
                nc.scalar.dma_start(
                    oat_sb[:, i * 2:(i + 1) * 2, :],
                    oatb_d[i * 2 * P:(i + 1) * 2 * P, :].rearrange(
                        "(c p) r -> p c r", p=P))
            for i in range(2):
                nc.scalar.dma_start(
                    oftb_sb[:, i * 2:(i + 1) * 2, :],
                    oftb_d[i * 2 * P:(i + 1) * 2 * P, :].rearrange(
                        "(c p) r -> p c r", p=P))

            # n3 via PE DoubleRow one-hot matmuls (runs during the gathers),
            # n1/n2 stream as their gathers land (explicit DVE waits on the
            # gather-completion semaphores; the preps only signal desc-gen)
            with (
                tc.tile_pool(name="pgo", bufs=1, space="PSUM") as pgo,
                tc.tile_pool(name="pst", bufs=1, space="PSUM") as pst,
                tc.tile_pool(name="pst2", bufs=2, space="PSUM") as pst2,
            ):
                for rb in range(RB):
                    r0 = rb * 512
                    stat3 = pst.tile([1, 512], f32, tag="s3",
                                     name=f"s3_{rb}")
                    for ccp in range(2):
                        og = pgo.tile([P, 2, 512], f32, tag="og",
                                      name=f"og{rb}_{ccp}")
                        for s in range(2):
                            cc = 2 * ccp + s
                            for i in range(2):
                                nc.tensor.matmul(
                                    og[:, s, :],
                                    lhsT=qg_sb[:, 2 * i:2 * i + 2,
                                               cc * P:(cc + 1) * P],
                                    rhs=oft8_sb[:, 2 * i:2 * i + 2,
                                                r0:r0 + 512],
                                    start=(i == 0), stop=(i == 1),
                                    perf_mode=DR)
                        pr3 = spool.tile([P, 2, 512], bf16, tag="pr3",
                                         name=f"pr3{rb}_{ccp}")
                        nc.vector.tensor_mul(
                            pr3[:], oat_sb[:, 2 * ccp:2 * ccp + 2,
                                           r0:r0 + 512], og[:])
                        for s in range(2):
                            nc.tensor.matmul(
                                stat3[:], lhsT=onesb[:], rhs=pr3[:, s, :],
                                start=(ccp == 0 and s == 0),
                                stop=(ccp == 1 and s == 1))
                    so3 = spool.tile([1, 512], f32, tag="so3",
                                     name=f"so3_{rb}")
                    nc.scalar.copy(so3[:], stat3[:])
                    nc.gpsimd.dma_start(nst_d[2, rb:rb + 1, :], so3[:])

                for slot, (mt, g_sb, nm, sem) in enumerate((
                    (oat_sb, ga_sb, "a", gsems[0]),
                    (oftb_sb, gf_sb, "f", gsems[1]),
                )):
                    nc.vector.wait_ge(sem, 16)
                    for rb in range(RB):
                        r0 = rb * 512
                        stat = pst2.tile([1, 512], f32, tag=f"s{nm}",
                                         name=f"s{nm}{rb}")
                        pr = spool.tile([P, 4, 512], bf16, tag=f"pr{nm}",
                                        name=f"pr{nm}{rb}")
                        nc.vector.tensor_mul(
                            pr[:], mt[:, :, r0:r0 + 512],
                            g_sb[:, :, r0:r0 + 512])
                        for cc in range(4):
                            nc.tensor.matmul(
                                stat[:], lhsT=onesb[:], rhs=pr[:, cc, :],
                                start=(cc == 0), stop=(cc == 3))
                        so = spool.tile([1, 512], f32, tag=f"so{nm}",
                                        name=f"so{nm}{rb}")
                        nc.scalar.copy(so[:], stat[:])
                        eng = nc.sync if slot == 0 else nc.gpsimd
                        eng.dma_start(nst_d[slot, rb:rb + 1, :], so[:])
    nc.finalize()
    return nc


# ---------------------------------------------------------------- host math
def _greedy_ext(sim):
    om, m = sim.shape
    used = np.zeros(m, dtype=bool)
    I = np.empty(om, dtype=np.int32)
    for r in range(om):
        row = np.where(used, -np.inf, sim[r])
        c = int(np.argmax(row))
        I[r] = c
        used[c] = True
    ext = np.empty(m, dtype=np.int32)
    ext[:om] = I
    ext[om:] = np.nonzero(~used)[0]
    return ext


def _triplet_weights(label, seq_len, vaf_avf):
    f32 = np.float32
    y = np.asarray(label).astype(np.int64)
    n_idx = np.nonzero(y == 0)[0]
    a_idx = np.nonzero(y == 1)[0]
    W = np.zeros((B, T, 4), f32)
    ar = np.arange(T)
    Nn, Na = len(n_idx), len(a_idx)
    if Nn and Na:
        for b in n_idx:
            L = int(seq_len[b])
            W[b, :, 0] = (ar < L).astype(f32) * WSCALE / (f32(L) * Nn)
        for b in a_idx:
            L = int(seq_len[b])
            k = L // 16 + 1
            sig = np.asarray(vaf_avf[b], np.float64)
            valid = ar < L
            o_s = np.argsort(np.where(valid, sig, np.inf), kind="stable")
            o_l = np.argsort(np.where(valid, -sig, np.inf), kind="stable")
            W[b, o_s[:k], 1] = WSCALE / (f32(k) * Na)
            W[b, o_l[:k], 2] = WSCALE / (f32(k) * Na)
    return W, Nn, Na


_runner_cache = {}


def _make_runner(nc):
    """Cached variant of bass2jax.run_bass_via_pjrt's multi-core path: jit
    once per program, reuse the compiled executable across kernel() calls."""
    import jax
    import numpy as _np
    from jax.experimental.shard_map import shard_map
    from jax.sharding import Mesh, PartitionSpec
    from concourse import bass2jax, mybir

    bass2jax.install_neuronx_cc_hook()
    assert nc.dbg_addr is None or not nc.dbg_callbacks
    partition_name = (nc.partition_id_tensor.name
                      if nc.partition_id_tensor else None)
    in_names, out_names, out_avals, zero_shapes = [], [], [], []
    for alloc in nc.m.functions[0].allocations:
        if not isinstance(alloc, mybir.MemoryLocationSet):
            continue
        name = alloc.memorylocations[0].name
        if alloc.kind == "ExternalInput":
            if name != partition_name:
                in_names.append(name)
        elif alloc.kind == "ExternalOutput":
            shape = tuple(alloc.tensor_shape)
            dtype = mybir.dt.np(alloc.dtype)
            out_names.append(name)
            out_avals.append(jax.core.ShapedArray(shape, dtype))
            zero_shapes.append((shape, dtype))
    n_params = len(in_names)
    n_outs = len(out_names)
    all_in = list(in_names) + list(out_names)
    if partition_name is not None:
        all_in.append(partition_name)
    donate = tuple(range(n_params, n_params + n_outs))

    def _body(*args):
        operands = list(args)
        if partition_name is not None:
            operands.append(bass2jax.partition_id_tensor())
        return tuple(bass2jax._bass_exec_p.bind(
            *operands,
            out_avals=tuple(out_avals),
            in_names=tuple(all_in),
            out_names=tuple(out_names),
            lowering_input_output_aliases=(),
            sim_require_finite=True,
            sim_require_nnan=True,
            nc=nc,
        ))

    devices = jax.devices()[:N_CORES]
    mesh = Mesh(_np.asarray(devices), ("core",))
    in_specs = (PartitionSpec("core"),) * (n_params + n_outs)
    out_specs = (PartitionSpec("core"),) * n_outs
    sharded = jax.jit(
        shard_map(_body, mesh=mesh, in_specs=in_specs, out_specs=out_specs,
                  check_rep=False),
        donate_argnums=donate, keep_unused=True)

    def run(in_maps):
        concat_in = [
            np.concatenate([np.asarray(m[name]) for m in in_maps], axis=0)
            for name in in_names
        ]
        concat_zeros = [
            np.zeros((N_CORES * s[0], *s[1:]), d) for (s, d) in zero_shapes
        ]
        out_arrs = sharded(*concat_in, *concat_zeros)
        return [
            {name: np.asarray(out_arrs[i]).reshape(
                N_CORES, *out_avals[i].shape)[c]
             for i, name in enumerate(out_names)}
            for c in range(N_CORES)
        ]

    return run


def _run_spmd(nc, in_maps):
    key = id(nc)
    if key not in _runner_cache:
        _runner_cache[key] = _make_runner(nc)
    return _runner_cache[key](in_maps)


def kernel(v_satt, va_satt, vf_satt, vaf_satt, v_avf, va_avf, vf_avf, vaf_avf,
           va_out, vf_out, vaf_out, lamda1, lamda2, lamda3, lamda4,
           label, seq_len):
    f32 = np.float32
    v8 = np.asarray(v_satt, f32).reshape(B * T, M).astype(F8)
    oa8 = np.asarray(va_satt, f32).reshape(B * T, OM).astype(F8)
    of8 = np.asarray(vf_satt, f32).reshape(B * T, OM).astype(F8)
    vaf8 = np.asarray(vaf_satt, f32).reshape(B * T, M).astype(F8)

    W, Nn, Na = _triplet_weights(label, seq_len, vaf_avf)
    w8 = np.zeros((B * T, P), F8)
    w8[:, 0:4] = W.reshape(B * T, 4).astype(F8)

    if "p1" not in _prog_cache:
        _prog_cache["p1"] = _build_pass1(RPC)
    if "p2" not in _prog_cache:
        _prog_cache["p2"] = _build_pass2(RPC)

    def sl(x, c):
        return x[c * RPC:(c + 1) * RPC]

    in1 = [
        dict(v8=sl(v8, c), oa8=sl(oa8, c), of8=sl(of8, c),
             vaf8=sl(vaf8, c), w8=sl(w8, c))
        for c in range(N_CORES)
    ]
    res1 = _run_spmd(_prog_cache["p1"], in1)

    G_A = np.zeros((OM, M), np.float64)
    G_F = np.zeros((OM, M), np.float64)
    Tm = np.zeros((4, M), np.float64)
    for r in res1:
        G_A += r["ga"].astype(np.float64)
        G_F += r["gf"].astype(np.float64)
        Tm += r["tm"]

    # norms: cheap O(n) scalar summaries, computed host-side from the same
    # fp8-rounded values the device consumes
    v8f = v8.astype(f32)
    oa8f = oa8.astype(f32)
    of8f = of8.astype(f32)
    sqV = np.square(v8f)
    nV = np.maximum(np.sqrt(sqV.sum(0)), 1e-12)
    rnV = np.sqrt(sqV.sum(1, dtype=np.float64))
    rnA = np.sqrt(np.square(oa8f).sum(1, dtype=np.float64))
    rnF = np.sqrt(np.square(of8f).sum(1, dtype=np.float64))
    extA = _greedy_ext((G_A / nV[None, :]).astype(f32))
    extF = _greedy_ext((G_F / nV[None, :]).astype(f32))

    # gather matrices: VgA[:, c] = V[:, invA[c]];  OFg[:, c] = OF[:, g[c]]
    invA = np.empty(M, np.int64)
    invA[extA] = np.arange(M)
    invF = np.empty(M, np.int64)
    invF[extF] = np.arange(M)
    QA = np.zeros((M, OM), F8)
    QA[invA[:OM], np.arange(OM)] = 1.0
    QF = np.zeros((M, OM), F8)
    QF[invF[:OM], np.arange(OM)] = 1.0
    g = extF[invA[:OM]]
    Qg = np.zeros((OM, OM), F8)
    selg = g < OM
    Qg[g[selg], np.arange(OM)[selg]] = 1.0

    vtb = np.ascontiguousarray(
        v8.astype(BF).reshape(N_CORES, RPC, M).transpose(0, 2, 1))
    oatb = np.ascontiguousarray(
        oa8.astype(BF).reshape(N_CORES, RPC, OM).transpose(0, 2, 1))
    oftb = np.ascontiguousarray(
        of8.astype(BF).reshape(N_CORES, RPC, OM).transpose(0, 2, 1))
    oft8 = np.ascontiguousarray(
        of8.reshape(N_CORES, RPC, OM).transpose(0, 2, 1))
    Qg = np.zeros((OM, OM), F8)
    selg = g < OM
    Qg[g[selg], np.arange(OM)[selg]] = 1.0

    def idx_tile(vals):
        ix16 = np.zeros((16, OM // 16), np.int16)
        for j in range(OM):
            ix16[j % 16, j // 16] = vals[j]
        return np.tile(ix16, (8, 1))  # replicated across the 8 Q7 cores

    ixa = idx_tile(invA[:OM])
    ixf = idx_tile(invF[:OM])

    in2 = [
        dict(vtb=vtb[c], oatb=oatb[c], oftb=oftb[c], oft8=oft8[c],
             qg8=Qg, ixa=ixa, ixf=ixf)
        for c in range(N_CORES)
    ]
    res2 = _run_spmd(_prog_cache["p2"], in2)
    nst = np.concatenate(
        [r["nst"].reshape(3, RPC) for r in res2], axis=1)  # [3, B*T]

    n1 = nst[0].astype(np.float64)
    n2 = nst[1].astype(np.float64)
    n3 = nst[2].astype(np.float64)

    def cos_term(num, rx, ry):
        den = np.maximum(rx * ry, 1e-8)
        return (1.0 - num / den).reshape(B, T).mean(1).sum()

    d_sum = (cos_term(n1, rnV, rnA) + cos_term(n2, rnV, rnF)
             + cos_term(n3, rnA, rnF)) / B

    ar = np.arange(T)
    seqm = (ar[None, :] < np.asarray(seq_len)[:, None]).astype(np.float64)
    Vs = np.asarray(v_avf, np.float64) * seqm
    As = np.asarray(va_avf, np.float64) * seqm
    Fs = np.asarray(vf_avf, np.float64) * seqm

    def ce(q, p):
        e = 1e-6
        q = np.clip(q, e, 1 - e)
        p = np.clip(p, e, 1 - e)
        return -(p * np.log(q) + (1 - p) * np.log(1 - q)).mean()

    ma_loss = d_sum + ce(Vs, As) + ce(Vs, Fs) + ce(As, Fs)

    yf = np.asarray(label).astype(np.float64)

    def bce(p, yy):
        p = np.asarray(p, np.float64)
        return -(yy * np.log(p) + (1 - yy) * np.log(1 - p)).mean()

    a_loss = bce(va_out, yf)
    f_loss = bce(vf_out, yf)
    raf_loss = bce(vaf_out, yf)

    if Nn == 0 or Na == 0:
        trip = 0.0
    else:
        anchor, pos, neg = Tm[0] / WSCALE, Tm[1] / WSCALE, Tm[2] / WSCALE
        nrm = lambda x: x / np.linalg.norm(x)
        a_, p_, g_ = nrm(anchor), nrm(pos), nrm(neg)
        d = lambda x, z: np.linalg.norm(x - z + 1e-6)
        trip = max(d(a_, p_) - d(a_, g_) + 5.0, 0.0)

    lam = [float(lamda1), float(lamda2), float(lamda3), float(lamda4)]
    total = (lam[0] * ma_loss + lam[1] * (a_loss + f_loss)
             + lam[2] * raf_loss + lam[3] * trip)
    return np.array([total, ma_loss, a_loss + f_loss, raf_loss, trip], f32)
